# revision 47
# baseline (speedup 1.0000x reference)
"""Multi-head attention (B=2, T=2048, H=1024, 16 heads) on 8 trn2 cores.

Sharding: data-parallel over batch (2) x tensor-parallel over head groups
(4 heads/core).  Each core computes qkv projection for its 4 heads,
attention, and a partial out-projection; the host sums 4 partials per
batch and adds b_out.

Key structural choices vs the straightforward version:
  * Host-side transpose: x arrives as xT [H, T] so no on-device PE
    transposes / DVE copies are needed to stage the projection rhs.
  * Mask-driven key compaction: the boolean mask kills ~half the keys
    exactly (exp(-1e9) == 0 in f32), so the host gathers only valid key
    rows into x_kv (padded to a multiple of 128).  Scores, exp and AV
    run on ~half the key dim; padded tail keys get a -1e9 bias so they
    contribute exactly 0, and their vp ones-column entry is 0.
  * V is produced directly in key-major orientation (lhsT = xT_kv
    chunk, rhs = W_v slice), so no V transpose either.
  * bf16 operands everywhere on the matmul paths (f32 PSUM accum),
    halving DMA and SBUF; maskbias/denominators stay f32.
  * Normalization: denominator row from the ones-augmented AV matmul,
    reciprocal_approx_fast on DVE, gpsimd partition_broadcast, DVE
    multiply.  No 3us single-lane reciprocal, no PE broadcast matmul.
  * Out-projection interleaved per 512-query block; a short burst of
    dummy warm-up matmuls at t=0 keeps the PE HAM clock from starting
    cold during the input DMA.
"""

import sys

sys.path.insert(0, "/opt/trn_rl_repo")

import numpy as np
from ml_dtypes import bfloat16

B, T, H = 2, 2048, 1024
NH, DK = 16, 64
HPC = 4           # heads per core
NCORES = 8
NB = T // 512     # query blocks
KT = H // 128     # contraction tiles for projections

_CACHE = {}


def _build(nv_pad):
    import concourse.bacc as bacc
    import concourse.mybir as mybir
    import concourse.tile as tile

    f32 = mybir.dt.float32
    bf16 = mybir.dt.bfloat16
    AF = mybir.ActivationFunctionType
    ALU = mybir.AluOpType

    NKT = nv_pad // 128   # key tiles

    nc = bacc.Bacc("TRN2", target_bir_lowering=False, debug=False)

    xT_d = nc.dram_tensor("xT", [H, T], bf16, kind="ExternalInput")
    xkvT_d = nc.dram_tensor("xkvT", [H, nv_pad], bf16, kind="ExternalInput")
    wq_d = nc.dram_tensor("w_qk", [H, 512], bf16, kind="ExternalInput")
    wv_d = nc.dram_tensor("w_v", [H, 256], bf16, kind="ExternalInput")
    wout_d = nc.dram_tensor("w_out", [2 * 128, H], bf16, kind="ExternalInput")
    maskb_d = nc.dram_tensor("maskbias", [128, NKT], f32, kind="ExternalInput")
    vones_d = nc.dram_tensor("validones", [128, NKT], f32, kind="ExternalInput")
    bqk_d = nc.dram_tensor("b_qk", [128, 4], f32, kind="ExternalInput")
    bvb_d = nc.dram_tensor("b_vb", [128, 256], f32, kind="ExternalInput")
    out_d = nc.dram_tensor("out_partial", [T, H], f32, kind="ExternalOutput")

    kv_blocks = [(o, min(512, nv_pad - o)) for o in range(0, nv_pad, 512)]

    with tile.TileContext(nc) as tc:
        with (
            tc.tile_pool(name="persist", bufs=1) as pp,
            tc.tile_pool(name="expp", bufs=4) as ep,
            tc.tile_pool(name="recp", bufs=4) as rp,
            tc.tile_pool(name="ostage", bufs=4) as osp,
            tc.tile_pool(name="psum", bufs=1, space="PSUM") as psp,
        ):
            # ---- persistent SBUF tiles ----
            scratch = pp.tile([128, 256], bf16, tag="scratch", name="scratch")
            nc.vector.memset(scratch, 0.125)
            wq = pp.tile([128, KT * 512], bf16, tag="wq", name="wq")
            wv = pp.tile([128, KT * 256], bf16, tag="wv", name="wv")
            wout = pp.tile([128, 2 * H], bf16, tag="wout", name="wout")
            maskb = pp.tile([128, NKT], f32, tag="maskb", name="maskb")
            vones = pp.tile([128, NKT], f32, tag="vones", name="vones")
            bqk = pp.tile([128, 4], f32, tag="bqk", name="bqk")
            bvb = pp.tile([128, 256], f32, tag="bvb", name="bvb")
            xkvT = pp.tile([128, KT * nv_pad], bf16, tag="xkvT", name="xkvT")
            xT = pp.tile([128, KT * T], bf16, tag="xT", name="xT")
            qT = [pp.tile([128, T], bf16, tag=f"qT{p}", name=f"qT{p}")
                  for p in range(2)]
            kT = [pp.tile([128, nv_pad], bf16, tag=f"kT{p}", name=f"kT{p}")
                  for p in range(2)]
            vp = [pp.tile([128, NKT * 65], bf16, tag=f"vp{h}", name=f"vp{h}")
                  for h in range(HPC)]
            attn = [pp.tile([128, T], bf16, tag=f"attn{p}", name=f"attn{p}")
                    for p in range(2)]
            # denominator staging at partitions {0,32} (SBUF AP start rule);
            # persistent + memset once so rows 1..31 are defined for the
            # batched reciprocal
            dd = pp.tile([33, 512], f32, tag="dd", name="dd")
            nc.vector.memset(dd, 1.0)
            rr = pp.tile([33, 512], f32, tag="rr", name="rr")

            # ---- PE warm-up: keep HAM busy while input DMAs run ----
            for i in range(40):
                wps = psp.tile([128, 1024], f32, tag="ss", bufs=2, name="wps")
                nc.tensor.matmul(
                    wps[:, 0:128], scratch[:, 0:128], scratch[:, 128:256],
                    start=True, stop=True,
                )

            # ---- input DMAs ----
            nc.gpsimd.dma_start(out=maskb, in_=maskb_d[:, :])
            nc.gpsimd.dma_start(out=vones, in_=vones_d[:, :])
            nc.gpsimd.dma_start(out=bqk, in_=bqk_d[:, :])
            nc.gpsimd.dma_start(out=bvb, in_=bvb_d[:, :])
            for kt in range(KT):
                nc.gpsimd.dma_start(
                    out=wq[:, kt * 512:(kt + 1) * 512],
                    in_=wq_d[kt * 128:(kt + 1) * 128, :],
                )
                nc.gpsimd.dma_start(
                    out=wv[:, kt * 256:(kt + 1) * 256],
                    in_=wv_d[kt * 128:(kt + 1) * 128, :],
                )
            for p in range(2):
                nc.gpsimd.dma_start(
                    out=wout[:, p * H:(p + 1) * H],
                    in_=wout_d[p * 128:(p + 1) * 128, :],
                )
            for kt in range(KT):
                nc.sync.dma_start(
                    out=xkvT[:, kt * nv_pad:(kt + 1) * nv_pad],
                    in_=xkvT_d[kt * 128:(kt + 1) * 128, :],
                )
            # xT in query-block-major pieces so q-proj for nb0 is ready
            # as early as possible
            for nb in range(NB):
                for kt in range(KT):
                    nc.sync.dma_start(
                        out=xT[:, kt * T + nb * 512: kt * T + (nb + 1) * 512],
                        in_=xT_d[kt * 128:(kt + 1) * 128,
                                 nb * 512:(nb + 1) * 512],
                    )

            # vp ones columns (0 for padded key rows)
            for h in range(HPC):
                vpv = vp[h].rearrange("p (t c) -> p t c", c=65)
                nc.gpsimd.tensor_copy(vpv[:, :, 64], vones)

            # ---- projections ----
            def proj_k_block(pair, o, w, scalar=False):
                # kT[pair][dk(128), keys o:o+w] = W_k.T @ x_kv
                mt = 2 + pair
                ps = psp.tile([128, 512], f32, tag="ps", bufs=2, name="ps")
                for kt in range(KT):
                    nc.tensor.matmul(
                        ps[:, 0:w],
                        wq[:, kt * 512 + mt * 128: kt * 512 + (mt + 1) * 128],
                        xkvT[:, kt * nv_pad + o: kt * nv_pad + o + w],
                        start=(kt == 0), stop=(kt == KT - 1),
                    )
                if scalar:
                    nc.scalar.activation(
                        kT[pair][:, o:o + w], ps[:, 0:w], AF.Identity,
                        bias=bqk[:, mt:mt + 1], scale=1.0)
                else:
                    nc.vector.tensor_scalar_add(
                        kT[pair][:, o:o + w], ps[:, 0:w], bqk[:, mt:mt + 1],
                    )

            def proj_q(pair, nb, scalar=False):
                mt = pair
                ps = psp.tile([128, 512], f32, tag="ps", bufs=2, name="ps")
                for kt in range(KT):
                    nc.tensor.matmul(
                        ps,
                        wq[:, kt * 512 + mt * 128: kt * 512 + (mt + 1) * 128],
                        xT[:, kt * T + nb * 512: kt * T + nb * 512 + 512],
                        start=(kt == 0), stop=(kt == KT - 1),
                    )
                if scalar:
                    nc.scalar.activation(
                        qT[pair][:, nb * 512:(nb + 1) * 512], ps, AF.Identity,
                        bias=bqk[:, mt:mt + 1], scale=1.0)
                else:
                    nc.vector.tensor_scalar_add(
                        qT[pair][:, nb * 512:(nb + 1) * 512], ps,
                        bqk[:, mt:mt + 1],
                    )

            def proj_v_kb(kb):
                # key-major: v[key, dk4] = x_kv @ W_v (all heads),
                # one keytile at a time
                ps = psp.tile([128, 512], f32, tag="ps", bufs=2, name="pv")
                for kt in range(KT):
                    nc.tensor.matmul(
                        ps[:, 0:256],
                        xkvT[:, kt * nv_pad + kb * 128:
                             kt * nv_pad + (kb + 1) * 128],
                        wv[:, kt * 256:(kt + 1) * 256],
                        start=(kt == 0), stop=(kt == KT - 1),
                    )
                for h in range(HPC):
                    nc.vector.tensor_tensor(
                        out=vp[h][:, kb * 65: kb * 65 + 64],
                        in0=ps[:, h * 64:(h + 1) * 64],
                        in1=bvb[:, h * 64:(h + 1) * 64],
                        op=ALU.add,
                    )

            # ---- attention for one pair, one query block ----
            # thunks: {kb: fn} fired at the TOP of that kb iteration —
            # before its score matmuls — so a thunk at kb may emit the
            # projection work that kb and later iterations depend on.
            def attention_nb(hp, nb, thunks=None):
                accs = [
                    psp.tile([65, 512], f32, tag="acc", bufs=2, name="acc")
                    for lh in range(2)
                ]
                for kb in range(NKT):
                    if thunks and kb in thunks:
                        for fn in thunks[kb]:
                            fn()
                    ss = psp.tile([128, 1024], f32, tag="ss", bufs=2, name="ss")
                    for lh in range(2):
                        r0 = lh * 64
                        nc.tensor.matmul(
                            ss[:, lh * 512:(lh + 1) * 512],
                            kT[hp][r0:r0 + 64, kb * 128:(kb + 1) * 128],
                            qT[hp][r0:r0 + 64, nb * 512:nb * 512 + 512],
                            start=True, stop=True,
                        )
                    ex = ep.tile([128, 1024], bf16, tag="ex", name="ex")
                    nc.scalar.activation(
                        ex, ss, AF.Exp,
                        bias=maskb[:, kb:kb + 1], scale=0.125,
                    )
                    for lh in range(2):
                        nc.tensor.matmul(
                            accs[lh],
                            vp[hp * 2 + lh][:, kb * 65: kb * 65 + 65],
                            ex[:, lh * 512:(lh + 1) * 512],
                            start=(kb == 0), stop=(kb == NKT - 1),
                        )
                # Normalization: denominators batched at partitions {0,32}
                # for one reciprocal (DVE cost ~ free size); unnormalized acc
                # copied out to free the PSUM slots early; broadcast and
                # multiply ride GPSIMD to keep the DVE queue short.
                uns = []
                for lh in range(2):
                    nc.vector.tensor_copy(
                        dd[32 * lh:32 * lh + 1, :], accs[lh][64:65, :])
                    un = rp.tile([64, 512], f32, tag=f"un{lh}", name="un")
                    nc.vector.tensor_copy(un, accs[lh][0:64, :])
                    uns.append(un)
                # (Ln->Exp reciprocal on ScalarE thrashes ACT table sets —
                # 17 ACT_TABLE_LOADs, exp 1111->1333ns — so 1/x stays on DVE)
                nc.vector.reciprocal(rr, dd)
                # partition_broadcast ucode requires a partition-0-based AP
                # (HW-verified: non-zero start partition reads garbage)
                rr1 = rp.tile([1, 512], f32, tag="rr1", name="rr1")
                nc.vector.tensor_copy(rr1, rr[32:33, :])
                for lh in range(2):
                    recb = rp.tile([64, 512], f32, tag=f"recb{lh}",
                                   name="recb")
                    nc.gpsimd.partition_broadcast(
                        recb, rr[0:1, :] if lh == 0 else rr1)
                    nc.vector.tensor_tensor(
                        out=attn[hp][lh * 64:(lh + 1) * 64,
                                     nb * 512:nb * 512 + 512],
                        in0=uns[lh],
                        in1=recb,
                        op=ALU.mult,
                    )

            def outproj_unit(mt, ob, tail=False):
                po = psp.tile([128, 512], f32, tag="ps", bufs=2, name="po")
                for p in range(2):
                    nc.tensor.matmul(
                        po,
                        attn[p][:, mt * 128:(mt + 1) * 128],
                        wout[:, p * H + ob * 512: p * H + ob * 512 + 512],
                        start=(p == 0), stop=(p == 1),
                    )
                ot = osp.tile([128, 512], f32, tag="ot", name="ot")
                # in the drain tail ScalarE is idle (exps done): split the
                # PSUM->SBUF staging across both engines to halve the tail
                if tail and ob == 1:
                    nc.scalar.copy(ot, po)
                else:
                    nc.vector.tensor_copy(ot, po)
                nc.sync.dma_start(
                    out=out_d[mt * 128:(mt + 1) * 128,
                              ob * 512:ob * 512 + 512],
                    in_=ot,
                )

            # ---- schedule ----
            # The first attention block only needs k01's first key block,
            # the first 4 V keytiles and q(0,0) — emit just those, then
            # start exp (~15us in) and feed the rest of the projections
            # through per-kb thunks sized to the PE's slack while ScalarE
            # paces the pipeline.  Out-projections lag one query block so
            # their PSUM->SBUF copies queue behind the next block's
            # denominator copies on DVE (which free the acc PSUM slots).
            def pk(pair, i):
                o, w = kv_blocks[i]
                return lambda: proj_k_block(pair, o, w)

            def pv(kb):
                return lambda: proj_v_kb(kb)

            def pq(pair, nb):
                return lambda: proj_q(pair, nb)

            proj_k_block(0, *kv_blocks[0])
            proj_v_kb(0)
            proj_q(0, 0)

            # v1-v3 feed in per-kb so the first exp fires ~24 matmuls
            # earlier; attention kb only ever needs V keytile kb
            t00 = {kb: [pv(kb)] for kb in range(1, min(4, NKT))}
            t00[4] = [pk(0, 1)] + [pv(kb) for kb in range(4, min(8, NKT))]
            if NKT > 8:
                t00[8] = [pk(0, 2)] + [pv(kb) for kb in range(8, NKT)]
                t00[8].append(pq(0, 1))
            else:
                t00[4].append(pq(0, 1))
            attention_nb(0, 0, thunks=t00)
            attention_nb(0, 1, thunks={3: [pk(1, 0)],
                                       6: [pq(0, 2)]})
            t02 = {3: [pk(1, 1)], 6: [pq(0, 3)]}
            if NKT > 8:
                t02[8] = [pk(1, 2)]
            attention_nb(0, 2, thunks=t02)
            attention_nb(0, 3, thunks={3: [pq(1, 0)], 6: [pq(1, 1)]})

            def outproj_nb(nb):
                for mt in range(4 * nb, 4 * nb + 4):
                    for ob in range(2):
                        outproj_unit(mt, ob)

            attention_nb(1, 0, thunks={3: [pq(1, 2)]})
            attention_nb(1, 1, thunks={3: [pq(1, 3)]})
            outproj_nb(0)
            attention_nb(1, 2)
            outproj_nb(1)
            attention_nb(1, 3)
            outproj_nb(2)
            for mt in range(12, 16):
                for ob in range(2):
                    outproj_unit(mt, ob, tail=True)

    nc.compile()
    return nc


def _get_nc(nv_pad):
    key = f"nc{nv_pad}"
    if key not in _CACHE:
        _CACHE[key] = _build(nv_pad)
    return _CACHE[key]


def _prep_in_maps(x, mask, W_qkv, b_qkv, W_out):
    """Returns (in_maps, nv_pad)."""
    idxs = [np.flatnonzero(mask[b, 0, 0, :] != 0) for b in range(B)]
    nvs = [len(i) for i in idxs]
    nv_pad = max(128, ((max(nvs) + 127) // 128) * 128)
    NKT = nv_pad // 128

    xTs, xkvTs, maskbs, voness = [], [], [], []
    for b in range(B):
        xTs.append(np.ascontiguousarray(x[b].T.astype(bfloat16)))
        xkv = np.zeros((nv_pad, H), dtype=np.float32)
        xkv[: nvs[b]] = x[b][idxs[b]]
        xkvTs.append(np.ascontiguousarray(xkv.T.astype(bfloat16)))
        mb = np.zeros(nv_pad, dtype=np.float32)
        mb[nvs[b]:] = -1e9
        maskbs.append(np.ascontiguousarray(mb.reshape(NKT, 128).T))
        vo = np.zeros(nv_pad, dtype=np.float32)
        vo[: nvs[b]] = 1.0
        voness.append(np.ascontiguousarray(vo.reshape(NKT, 128).T))

    in_maps = []
    for c in range(NCORES):
        b = c // 4
        h0 = (c % 4) * HPC
        # col order q01 | q23 | k01 | k23 (128 each)
        qk_cols = np.concatenate([
            np.arange(h0 * DK, (h0 + 2) * DK),
            np.arange((h0 + 2) * DK, (h0 + 4) * DK),
            np.arange(H + h0 * DK, H + (h0 + 2) * DK),
            np.arange(H + (h0 + 2) * DK, H + (h0 + 4) * DK),
        ])
        w_qk = np.ascontiguousarray(W_qkv[:, qk_cols].astype(bfloat16))
        v_cols = np.arange(2 * H + h0 * DK, 2 * H + (h0 + 4) * DK)
        w_v = np.ascontiguousarray(W_qkv[:, v_cols].astype(bfloat16))
        w_out = np.ascontiguousarray(
            W_out[h0 * DK:(h0 + 4) * DK, :].astype(bfloat16))
        b_qk = np.ascontiguousarray(
            b_qkv[qk_cols].reshape(4, 128).T.astype(np.float32))
        b_vb = np.ascontiguousarray(np.broadcast_to(
            b_qkv[v_cols].astype(np.float32), (128, 256)))
        in_maps.append({
            "xT": xTs[b],
            "xkvT": xkvTs[b],
            "w_qk": w_qk,
            "w_v": w_v,
            "w_out": w_out,
            "maskbias": maskbs[b],
            "validones": voness[b],
            "b_qk": b_qk,
            "b_vb": b_vb,
        })
    return in_maps, nv_pad


def _combine(partials, b_out):
    out = np.empty((B, T, H), dtype=np.float32)
    for b in range(B):
        acc = partials[4 * b].astype(np.float32)
        for i in range(1, 4):
            acc = acc + partials[4 * b + i]
        out[b] = acc + b_out[None, :]
    return out


def kernel(x, mask, W_qkv, b_qkv, W_out, b_out):
    x = np.asarray(x, dtype=np.float32)
    mask = np.asarray(mask)
    W_qkv = np.asarray(W_qkv, dtype=np.float32)
    b_qkv = np.asarray(b_qkv, dtype=np.float32)
    W_out = np.asarray(W_out, dtype=np.float32)
    b_out = np.asarray(b_out, dtype=np.float32)

    in_maps, nv_pad = _prep_in_maps(x, mask, W_qkv, b_qkv, W_out)
    nc = _get_nc(nv_pad)

    from concourse.bass_utils import run_bass_kernel_spmd

    res = run_bass_kernel_spmd(nc, in_maps, list(range(NCORES)))
    partials = [res.results[c]["out_partial"] for c in range(NCORES)]
    return _combine(partials, b_out)


# revision 49
# speedup vs baseline: 1.0262x; 1.0262x over previous
"""Multi-head attention (B=2, T=2048, H=1024, 16 heads) on 8 trn2 cores.

Sharding: data-parallel over batch (2) x tensor-parallel over head groups
(4 heads/core).  Each core computes qkv projection for its 4 heads,
attention, and a partial out-projection; the host sums 4 partials per
batch and adds b_out.

Key structural choices vs the straightforward version:
  * Host-side transpose: x arrives as xT [H, T] so no on-device PE
    transposes / DVE copies are needed to stage the projection rhs.
  * Mask-driven key compaction: the boolean mask kills ~half the keys
    exactly (exp(-1e9) == 0 in f32), so the host gathers only valid key
    rows into x_kv (padded to a multiple of 128).  Scores, exp and AV
    run on ~half the key dim; padded tail keys get a -1e9 bias so they
    contribute exactly 0, and their vp ones-column entry is 0.
  * V is produced directly in key-major orientation (lhsT = xT_kv
    chunk, rhs = W_v slice), so no V transpose either.
  * bf16 operands everywhere on the matmul paths (f32 PSUM accum),
    halving DMA and SBUF; maskbias/denominators stay f32.
  * Normalization: denominator row from the ones-augmented AV matmul,
    reciprocal_approx_fast on DVE, gpsimd partition_broadcast, DVE
    multiply.  No 3us single-lane reciprocal, no PE broadcast matmul.
  * Out-projection interleaved per 512-query block; a short burst of
    dummy warm-up matmuls at t=0 keeps the PE HAM clock from starting
    cold during the input DMA.
"""

import sys

sys.path.insert(0, "/opt/trn_rl_repo")

import numpy as np
from ml_dtypes import bfloat16

B, T, H = 2, 2048, 1024
NH, DK = 16, 64
HPC = 4           # heads per core
NCORES = 8
NB = T // 512     # query blocks
KT = H // 128     # contraction tiles for projections

_CACHE = {}


def _build(nv_pad):
    import concourse.bacc as bacc
    import concourse.mybir as mybir
    import concourse.tile as tile

    f32 = mybir.dt.float32
    bf16 = mybir.dt.bfloat16
    AF = mybir.ActivationFunctionType
    ALU = mybir.AluOpType

    NKT = nv_pad // 128   # key tiles

    nc = bacc.Bacc("TRN2", target_bir_lowering=False, debug=False)

    xT_d = nc.dram_tensor("xT", [H, T], bf16, kind="ExternalInput")
    xkvT_d = nc.dram_tensor("xkvT", [H, nv_pad], bf16, kind="ExternalInput")
    wq_d = nc.dram_tensor("w_qk", [H, 512], bf16, kind="ExternalInput")
    wv_d = nc.dram_tensor("w_v", [H, 256], bf16, kind="ExternalInput")
    wout_d = nc.dram_tensor("w_out", [2 * 128, H], bf16, kind="ExternalInput")
    maskb_d = nc.dram_tensor("maskbias", [128, NKT], f32, kind="ExternalInput")
    vones_d = nc.dram_tensor("validones", [128, NKT], f32, kind="ExternalInput")
    bqk_d = nc.dram_tensor("b_qk", [128, 4], f32, kind="ExternalInput")
    bvb_d = nc.dram_tensor("b_vb", [128, 256], f32, kind="ExternalInput")
    out_d = nc.dram_tensor("out_partial", [T, H], f32, kind="ExternalOutput")

    kv_blocks = [(o, min(512, nv_pad - o)) for o in range(0, nv_pad, 512)]

    with tile.TileContext(nc) as tc:
        with (
            tc.tile_pool(name="persist", bufs=1) as pp,
            tc.tile_pool(name="expp", bufs=4) as ep,
            tc.tile_pool(name="recp", bufs=4) as rp,
            tc.tile_pool(name="ostage", bufs=4) as osp,
            tc.tile_pool(name="psum", bufs=1, space="PSUM") as psp,
        ):
            # ---- persistent SBUF tiles ----
            scratch = pp.tile([128, 256], bf16, tag="scratch", name="scratch")
            nc.vector.memset(scratch, 0.125)
            wq = pp.tile([128, KT * 512], bf16, tag="wq", name="wq")
            wv = pp.tile([128, KT * 256], bf16, tag="wv", name="wv")
            wout = pp.tile([128, 2 * H], bf16, tag="wout", name="wout")
            maskb = pp.tile([128, NKT], f32, tag="maskb", name="maskb")
            vones = pp.tile([128, NKT], f32, tag="vones", name="vones")
            bqk = pp.tile([128, 4], f32, tag="bqk", name="bqk")
            bvb = pp.tile([128, 256], f32, tag="bvb", name="bvb")
            xkvT = pp.tile([128, KT * nv_pad], bf16, tag="xkvT", name="xkvT")
            xT = pp.tile([128, KT * T], bf16, tag="xT", name="xT")
            qT = [pp.tile([128, T], bf16, tag=f"qT{p}", name=f"qT{p}")
                  for p in range(2)]
            kT = [pp.tile([128, nv_pad], bf16, tag=f"kT{p}", name=f"kT{p}")
                  for p in range(2)]
            vp = [pp.tile([128, NKT * 65], bf16, tag=f"vp{h}", name=f"vp{h}")
                  for h in range(HPC)]
            attn = [pp.tile([128, T], bf16, tag=f"attn{p}", name=f"attn{p}")
                    for p in range(2)]
            # denominator staging at partitions {0,32} (SBUF AP start rule);
            # persistent + memset once so rows 1..31 are defined for the
            # batched reciprocal
            dd = pp.tile([33, 512], f32, tag="dd", name="dd")
            nc.vector.memset(dd, 1.0)
            rr = pp.tile([33, 512], f32, tag="rr", name="rr")

            # ---- PE warm-up: keep HAM busy while input DMAs run ----
            for i in range(40):
                wps = psp.tile([128, 1024], f32, tag="ss", bufs=2, name="wps")
                nc.tensor.matmul(
                    wps[:, 0:128], scratch[:, 0:128], scratch[:, 128:256],
                    start=True, stop=True,
                )

            # ---- input DMAs ----
            nc.gpsimd.dma_start(out=maskb, in_=maskb_d[:, :])
            nc.gpsimd.dma_start(out=vones, in_=vones_d[:, :])
            nc.gpsimd.dma_start(out=bqk, in_=bqk_d[:, :])
            nc.gpsimd.dma_start(out=bvb, in_=bvb_d[:, :])
            for kt in range(KT):
                nc.gpsimd.dma_start(
                    out=wq[:, kt * 512:(kt + 1) * 512],
                    in_=wq_d[kt * 128:(kt + 1) * 128, :],
                )
                nc.gpsimd.dma_start(
                    out=wv[:, kt * 256:(kt + 1) * 256],
                    in_=wv_d[kt * 128:(kt + 1) * 128, :],
                )
            for p in range(2):
                nc.gpsimd.dma_start(
                    out=wout[:, p * H:(p + 1) * H],
                    in_=wout_d[p * 128:(p + 1) * 128, :],
                )
            for kt in range(KT):
                nc.sync.dma_start(
                    out=xkvT[:, kt * nv_pad:(kt + 1) * nv_pad],
                    in_=xkvT_d[kt * 128:(kt + 1) * 128, :],
                )
            # xT in query-block-major pieces so q-proj for nb0 is ready
            # as early as possible
            for nb in range(NB):
                for kt in range(KT):
                    nc.sync.dma_start(
                        out=xT[:, kt * T + nb * 512: kt * T + (nb + 1) * 512],
                        in_=xT_d[kt * 128:(kt + 1) * 128,
                                 nb * 512:(nb + 1) * 512],
                    )

            # vp ones columns (0 for padded key rows)
            for h in range(HPC):
                vpv = vp[h].rearrange("p (t c) -> p t c", c=65)
                nc.gpsimd.tensor_copy(vpv[:, :, 64], vones)

            # ---- projections ----
            def proj_k_block(pair, o, w, scalar=False):
                # kT[pair][dk(128), keys o:o+w] = W_k.T @ x_kv
                mt = 2 + pair
                ps = psp.tile([128, 512], f32, tag="ps", bufs=2, name="ps")
                for kt in range(KT):
                    nc.tensor.matmul(
                        ps[:, 0:w],
                        wq[:, kt * 512 + mt * 128: kt * 512 + (mt + 1) * 128],
                        xkvT[:, kt * nv_pad + o: kt * nv_pad + o + w],
                        start=(kt == 0), stop=(kt == KT - 1),
                    )
                if scalar:
                    nc.scalar.activation(
                        kT[pair][:, o:o + w], ps[:, 0:w], AF.Identity,
                        bias=bqk[:, mt:mt + 1], scale=1.0)
                else:
                    nc.vector.tensor_scalar_add(
                        kT[pair][:, o:o + w], ps[:, 0:w], bqk[:, mt:mt + 1],
                    )

            def proj_q(pair, nb, scalar=False):
                mt = pair
                ps = psp.tile([128, 512], f32, tag="ps", bufs=2, name="ps")
                for kt in range(KT):
                    nc.tensor.matmul(
                        ps,
                        wq[:, kt * 512 + mt * 128: kt * 512 + (mt + 1) * 128],
                        xT[:, kt * T + nb * 512: kt * T + nb * 512 + 512],
                        start=(kt == 0), stop=(kt == KT - 1),
                    )
                if scalar:
                    nc.scalar.activation(
                        qT[pair][:, nb * 512:(nb + 1) * 512], ps, AF.Identity,
                        bias=bqk[:, mt:mt + 1], scale=1.0)
                else:
                    nc.vector.tensor_scalar_add(
                        qT[pair][:, nb * 512:(nb + 1) * 512], ps,
                        bqk[:, mt:mt + 1],
                    )

            def proj_v_kb(kb):
                # key-major: v[key, dk4] = x_kv @ W_v (all heads),
                # one keytile at a time
                ps = psp.tile([128, 512], f32, tag="ps", bufs=2, name="pv")
                for kt in range(KT):
                    nc.tensor.matmul(
                        ps[:, 0:256],
                        xkvT[:, kt * nv_pad + kb * 128:
                             kt * nv_pad + (kb + 1) * 128],
                        wv[:, kt * 256:(kt + 1) * 256],
                        start=(kt == 0), stop=(kt == KT - 1),
                    )
                for h in range(HPC):
                    nc.vector.tensor_tensor(
                        out=vp[h][:, kb * 65: kb * 65 + 64],
                        in0=ps[:, h * 64:(h + 1) * 64],
                        in1=bvb[:, h * 64:(h + 1) * 64],
                        op=ALU.add,
                    )

            # ---- attention for one pair, one query block ----
            # thunks: {kb: fn} fired at the TOP of that kb iteration —
            # before its score matmuls — so a thunk at kb may emit the
            # projection work that kb and later iterations depend on.
            def attention_nb(hp, nb, thunks=None):
                accs = [
                    psp.tile([65, 512], f32, tag="acc", bufs=2, name="acc")
                    for lh in range(2)
                ]
                for kb in range(NKT):
                    if thunks and kb in thunks:
                        for fn in thunks[kb]:
                            fn()
                    ss = psp.tile([128, 1024], f32, tag="ss", bufs=2, name="ss")
                    for lh in range(2):
                        r0 = lh * 64
                        nc.tensor.matmul(
                            ss[:, lh * 512:(lh + 1) * 512],
                            kT[hp][r0:r0 + 64, kb * 128:(kb + 1) * 128],
                            qT[hp][r0:r0 + 64, nb * 512:nb * 512 + 512],
                            start=True, stop=True,
                        )
                    ex = ep.tile([128, 1024], bf16, tag="ex", name="ex")
                    nc.scalar.activation(
                        ex, ss, AF.Exp,
                        bias=maskb[:, kb:kb + 1], scale=0.125,
                    )
                    for lh in range(2):
                        nc.tensor.matmul(
                            accs[lh],
                            vp[hp * 2 + lh][:, kb * 65: kb * 65 + 65],
                            ex[:, lh * 512:(lh + 1) * 512],
                            start=(kb == 0), stop=(kb == NKT - 1),
                        )
                # Normalization: denominators batched at partitions {0,32}
                # for one reciprocal (DVE cost ~ free size); unnormalized acc
                # copied out to free the PSUM slots early; broadcast and
                # multiply ride GPSIMD to keep the DVE queue short.
                uns = []
                for lh in range(2):
                    nc.vector.tensor_copy(
                        dd[32 * lh:32 * lh + 1, :], accs[lh][64:65, :])
                    un = rp.tile([64, 512], f32, tag=f"un{lh}", name="un")
                    nc.vector.tensor_copy(un, accs[lh][0:64, :])
                    uns.append(un)
                # (Ln->Exp reciprocal on ScalarE thrashes ACT table sets —
                # 17 ACT_TABLE_LOADs, exp 1111->1333ns — so 1/x stays on DVE)
                nc.vector.reciprocal(rr, dd)
                # partition_broadcast ucode requires a partition-0-based AP
                # (HW-verified: non-zero start partition reads garbage)
                rr1 = rp.tile([1, 512], f32, tag="rr1", name="rr1")
                nc.vector.tensor_copy(rr1, rr[32:33, :])
                for lh in range(2):
                    recb = rp.tile([64, 512], f32, tag=f"recb{lh}",
                                   name="recb")
                    nc.gpsimd.partition_broadcast(
                        recb, rr[0:1, :] if lh == 0 else rr1)
                    nc.vector.tensor_tensor(
                        out=attn[hp][lh * 64:(lh + 1) * 64,
                                     nb * 512:nb * 512 + 512],
                        in0=uns[lh],
                        in1=recb,
                        op=ALU.mult,
                    )

            def outproj_unit(mt, ob, tail=False):
                po = psp.tile([128, 512], f32, tag="ps", bufs=2, name="po")
                for p in range(2):
                    nc.tensor.matmul(
                        po,
                        attn[p][:, mt * 128:(mt + 1) * 128],
                        wout[:, p * H + ob * 512: p * H + ob * 512 + 512],
                        start=(p == 0), stop=(p == 1),
                    )
                ot = osp.tile([128, 512], f32, tag="ot", name="ot")
                # in the drain tail ScalarE is idle (exps done): split the
                # PSUM->SBUF staging across both engines to halve the tail
                if tail and ob == 1:
                    nc.scalar.copy(ot, po)
                else:
                    nc.vector.tensor_copy(ot, po)
                nc.sync.dma_start(
                    out=out_d[mt * 128:(mt + 1) * 128,
                              ob * 512:ob * 512 + 512],
                    in_=ot,
                )

            # ---- schedule ----
            # The first attention block only needs k01's first key block,
            # the first 4 V keytiles and q(0,0) — emit just those, then
            # start exp (~15us in) and feed the rest of the projections
            # through per-kb thunks sized to the PE's slack while ScalarE
            # paces the pipeline.  Out-projections lag one query block so
            # their PSUM->SBUF copies queue behind the next block's
            # denominator copies on DVE (which free the acc PSUM slots).
            def pk(pair, i):
                o, w = kv_blocks[i]
                return lambda: proj_k_block(pair, o, w)

            def pv(kb):
                return lambda: proj_v_kb(kb)

            def pq(pair, nb):
                return lambda: proj_q(pair, nb)

            proj_k_block(0, *kv_blocks[0])
            for kb in range(min(4, NKT)):
                proj_v_kb(kb)
            proj_q(0, 0)

            t00 = {4: [pk(0, 1)] + [pv(kb) for kb in range(4, min(8, NKT))]}
            if NKT > 8:
                t00[8] = [pk(0, 2)] + [pv(kb) for kb in range(8, NKT)]
                t00[8].append(pq(0, 1))
            else:
                t00[4].append(pq(0, 1))
            attention_nb(0, 0, thunks=t00)
            attention_nb(0, 1, thunks={3: [pk(1, 0)],
                                       6: [pq(0, 2)]})
            t02 = {3: [pk(1, 1)], 6: [pq(0, 3)]}
            if NKT > 8:
                t02[8] = [pk(1, 2)]
            attention_nb(0, 2, thunks=t02)
            attention_nb(0, 3, thunks={3: [pq(1, 0)], 6: [pq(1, 1)]})

            def outproj_nb(nb):
                for mt in range(4 * nb, 4 * nb + 4):
                    for ob in range(2):
                        outproj_unit(mt, ob)

            attention_nb(1, 0, thunks={3: [pq(1, 2)]})
            attention_nb(1, 1, thunks={3: [pq(1, 3)]})
            outproj_nb(0)
            attention_nb(1, 2)
            outproj_nb(1)
            attention_nb(1, 3)
            # both trailing out-projection blocks run after the last exp:
            # ScalarE is idle, so split their staging copies across engines
            for mt in range(8, 16):
                for ob in range(2):
                    outproj_unit(mt, ob, tail=True)

    nc.compile()
    return nc


def _get_nc(nv_pad):
    key = f"nc{nv_pad}"
    if key not in _CACHE:
        _CACHE[key] = _build(nv_pad)
    return _CACHE[key]


def _prep_in_maps(x, mask, W_qkv, b_qkv, W_out):
    """Returns (in_maps, nv_pad)."""
    idxs = [np.flatnonzero(mask[b, 0, 0, :] != 0) for b in range(B)]
    nvs = [len(i) for i in idxs]
    nv_pad = max(128, ((max(nvs) + 127) // 128) * 128)
    NKT = nv_pad // 128

    xTs, xkvTs, maskbs, voness = [], [], [], []
    for b in range(B):
        xTs.append(np.ascontiguousarray(x[b].T.astype(bfloat16)))
        xkv = np.zeros((nv_pad, H), dtype=np.float32)
        xkv[: nvs[b]] = x[b][idxs[b]]
        xkvTs.append(np.ascontiguousarray(xkv.T.astype(bfloat16)))
        mb = np.zeros(nv_pad, dtype=np.float32)
        mb[nvs[b]:] = -1e9
        maskbs.append(np.ascontiguousarray(mb.reshape(NKT, 128).T))
        vo = np.zeros(nv_pad, dtype=np.float32)
        vo[: nvs[b]] = 1.0
        voness.append(np.ascontiguousarray(vo.reshape(NKT, 128).T))

    in_maps = []
    for c in range(NCORES):
        b = c // 4
        h0 = (c % 4) * HPC
        # col order q01 | q23 | k01 | k23 (128 each)
        qk_cols = np.concatenate([
            np.arange(h0 * DK, (h0 + 2) * DK),
            np.arange((h0 + 2) * DK, (h0 + 4) * DK),
            np.arange(H + h0 * DK, H + (h0 + 2) * DK),
            np.arange(H + (h0 + 2) * DK, H + (h0 + 4) * DK),
        ])
        w_qk = np.ascontiguousarray(W_qkv[:, qk_cols].astype(bfloat16))
        v_cols = np.arange(2 * H + h0 * DK, 2 * H + (h0 + 4) * DK)
        w_v = np.ascontiguousarray(W_qkv[:, v_cols].astype(bfloat16))
        w_out = np.ascontiguousarray(
            W_out[h0 * DK:(h0 + 4) * DK, :].astype(bfloat16))
        b_qk = np.ascontiguousarray(
            b_qkv[qk_cols].reshape(4, 128).T.astype(np.float32))
        b_vb = np.ascontiguousarray(np.broadcast_to(
            b_qkv[v_cols].astype(np.float32), (128, 256)))
        in_maps.append({
            "xT": xTs[b],
            "xkvT": xkvTs[b],
            "w_qk": w_qk,
            "w_v": w_v,
            "w_out": w_out,
            "maskbias": maskbs[b],
            "validones": voness[b],
            "b_qk": b_qk,
            "b_vb": b_vb,
        })
    return in_maps, nv_pad


def _combine(partials, b_out):
    out = np.empty((B, T, H), dtype=np.float32)
    for b in range(B):
        acc = partials[4 * b].astype(np.float32)
        for i in range(1, 4):
            acc = acc + partials[4 * b + i]
        out[b] = acc + b_out[None, :]
    return out


def kernel(x, mask, W_qkv, b_qkv, W_out, b_out):
    x = np.asarray(x, dtype=np.float32)
    mask = np.asarray(mask)
    W_qkv = np.asarray(W_qkv, dtype=np.float32)
    b_qkv = np.asarray(b_qkv, dtype=np.float32)
    W_out = np.asarray(W_out, dtype=np.float32)
    b_out = np.asarray(b_out, dtype=np.float32)

    in_maps, nv_pad = _prep_in_maps(x, mask, W_qkv, b_qkv, W_out)
    nc = _get_nc(nv_pad)

    from concourse.bass_utils import run_bass_kernel_spmd

    res = run_bass_kernel_spmd(nc, in_maps, list(range(NCORES)))
    partials = [res.results[c]["out_partial"] for c in range(NCORES)]
    return _combine(partials, b_out)
